# revision 1
# baseline (speedup 1.0000x reference)
"""Trainium2 Bass kernel for nn_EquivariantProductBasisBlock (MACE product-basis block).

Self-contained: host-side sharding/preprocessing + Bass/Tile device kernel on 8 cores.

Math (validated vs reference): per node n, channel c, species s, x = x[n,c,:] in R^9:
    out[z] = sum_i x_i * F[z,i],   F = C1 + C2 @ x + C3h @ y,   y = {x_j x_k}_{j<=k}
with C* the species/channel coefficient tables folded from (u*, w*) on the host.
Then gate = f0 @ gate_kernel[s] + gate_bias[s]; f0*=gate[:C]; f1*=gate[C:];
out = [f0 @ lin0, f1 @ lin1] / sqrt(C).

Device layout: nodes on SBUF partitions (128/tile), species-sorted so every tile is
species-pure; schedule identical across cores (tile t = species t, 10 tiles/core).
Per tile: DVE builds monomial slots YX[128n,(c,56)]; PE transposes each channel-pair
slice -> V[112,128]; matmul(lhsT=V, rhs=Wblk[s,pair]) -> F^T[128n,72] in PSUM;
DVE multiplies by x (broadcast AP) and segment-reduces -> f[128n,(c,4)];
then gate matmul + gating + linear + output transposes.

Species overflow beyond 1024 nodes/species (and any species with >1024 nodes) is
computed on the host in numpy and merged (tiny for the target distribution).
"""

import numpy as np

N_CORES = 8
C, D, S = 128, 9, 10
NM = 45           # deg-2 monomials
NROW = 56         # per-channel slots: 45 y | 9 x | 1 one | 1 pad
KP = 2 * NROW     # 112 rows per channel-pair
NPAIR = C // 2    # 64
TILE_N = 128
TPC = S           # tiles per core (one per species)
NODES_PER_CORE = TPC * TILE_N          # 1280
CAP_PER_SPECIES = N_CORES * TILE_N     # 1024 device-handled nodes per species

# monomials ordered by diagonal offset o=k-j then j: slot(o,j) = OSTART[o]+j.
# This makes each device y-build op pure step-1 (out/in0/in1 all contiguous runs).
OSTART = [0] * D
for o in range(1, D):
    OSTART[o] = OSTART[o - 1] + (D - (o - 1))
MONO_JK = [(j, j + o) for o in range(D) for j in range(D - o)]


# ----------------------------------------------------------------------------
# host math
# ----------------------------------------------------------------------------

def _build_xr(node_feats):
    n = node_feats.shape[0]
    x = np.empty((n, C, D), np.float32)
    x[:, :, 0] = node_feats[:, :C]
    x[:, :, 1:4] = node_feats[:, C:4 * C].reshape(n, C, 3)
    x[:, :, 4:9] = node_feats[:, 4 * C:].reshape(n, C, 5)
    return x


def _build_coeff_tables(i):
    def c3h(u3, w3):
        c3 = np.einsum('zijkp,spc->sczijk', u3, w3[:, :, :], optimize=True)
        out = np.zeros(c3.shape[:4] + (NM,), np.float64)
        for m, (j, k) in enumerate(MONO_JK):
            out[..., m] = c3[..., j, k] if j == k else c3[..., j, k] + c3[..., k, j]
        return out

    def c2(u2, w2):
        return np.einsum('zijp,spc->sczij', u2, w2, optimize=True)

    def c1(u1, w1):
        return np.einsum('zip,spc->sczi', u1, w1, optimize=True)

    h0 = c3h(i['u3_0e'], i['w3_0e']); h1 = c3h(i['u3_1o'], i['w3_1o'])
    q0 = c2(i['u2_0e'], i['w2_0e']);  q1 = c2(i['u2_1o'], i['w2_1o'])
    l0 = c1(i['u1_0e'], i['w1_0e']);  l1 = c1(i['u1_1o'], i['w1_1o'])

    W = np.zeros((S, C, NROW, 36), np.float64)
    W[:, :, 0:45, 0:9] = np.moveaxis(h0[:, :, 0], -1, -2)
    W[:, :, 45:54, 0:9] = np.moveaxis(q0[:, :, 0], -1, -2)
    W[:, :, 54, 0:9] = l0[:, :, 0]
    for z in range(3):
        sl = slice(9 + z * 9, 18 + z * 9)
        W[:, :, 0:45, sl] = np.moveaxis(h1[:, :, z], -1, -2)
        W[:, :, 45:54, sl] = np.moveaxis(q1[:, :, z], -1, -2)
        W[:, :, 54, sl] = l1[:, :, z]
    return W.astype(np.float32)   # [S, C, 56, 36]


def _numpy_forward(inputs, idx):
    """Reference-equivalent host computation for node subset idx (overflow path)."""
    i = {k: np.asarray(v) for k, v in inputs.items()}
    nf = i['node_feats'][idx]; sp = i['node_species'][idx]
    xr = _build_xr(nf)
    W = _build_coeff_tables(i)
    n = nf.shape[0]
    V = np.empty((n, C, NROW), np.float32)
    for m, (j, k) in enumerate(MONO_JK):
        V[:, :, m] = xr[:, :, j] * xr[:, :, k]
    V[:, :, 45:54] = xr
    V[:, :, 54] = 1.0
    V[:, :, 55] = 0.0
    F = np.einsum('ncm,ncmz->ncz', V, W[sp], optimize=True)
    f = np.einsum('nczi,nci->ncz', F.reshape(n, C, 4, D), xr, optimize=True)
    f0, f1 = f[:, :, 0], f[:, :, 1:4]
    gate = np.einsum('nc,nck->nk', f0, i['gate_kernel'][sp], optimize=True) + i['gate_bias'][sp]
    f0g = f0 * gate[:, :C]
    f1g = f1 * gate[:, C:, None]
    inv = 1.0 / np.sqrt(np.float32(C))
    o0 = np.einsum('nc,ck->nk', f0g, i['lin_w_0e'], optimize=True) * inv
    o1 = np.einsum('ncd,ck->nkd', f1g, i['lin_w_1o'], optimize=True) * inv
    return np.concatenate([o0.reshape(n, C), o1.reshape(n, C * 3)], axis=1).astype(np.float32)


def _bf16(x):
    import ml_dtypes
    return np.asarray(x, np.float32).astype(ml_dtypes.bfloat16)


def host_prepare(inputs):
    """Returns (per_core_inmaps, scatter_idx [N_CORES,1280] global node ids (-1 pad),
    overflow_idx)."""
    i = {k: np.asarray(v) for k, v in inputs.items()}
    sp = i['node_species']
    ntot = sp.shape[0]

    order = np.argsort(sp, kind='stable')
    sorted_sp = sp[order]
    device_rows = np.full((N_CORES, NODES_PER_CORE), -1, np.int64)
    overflow = []
    for s in range(S):
        ids = order[sorted_sp == s]
        dev = ids[:CAP_PER_SPECIES]
        overflow.append(ids[CAP_PER_SPECIES:])
        for k in range(N_CORES):
            chunk = dev[k * TILE_N:(k + 1) * TILE_N]
            device_rows[k, s * TILE_N: s * TILE_N + len(chunk)] = chunk
    overflow_idx = np.concatenate(overflow) if overflow else np.zeros(0, np.int64)

    xr = _build_xr(i['node_feats'])                       # [N, C, 9]
    W = _build_coeff_tables(i)                            # [S, C, 56, 36]

    # block-diag pair table: [112 rows, S*64*72 cols] (row-major per pair block)
    wblk = np.zeros((KP, S * NPAIR * 72), np.float32)
    for s in range(S):
        for p in range(NPAIR):
            base = (s * NPAIR + p) * 72
            wblk[0:NROW, base:base + 36] = W[s, 2 * p]
            wblk[NROW:KP, base + 36:base + 72] = W[s, 2 * p + 1]

    gk = np.zeros((C, S * 2 * C), np.float32)             # rows c, col s*256 + j
    for s in range(S):
        gk[:, s * 256:(s + 1) * 256] = i['gate_kernel'][s]

    bias = np.zeros((C, S * 2), np.float32)               # rows k2%128, col s*2 + half
    for s in range(S):
        bias[:, 2 * s] = i['gate_bias'][s, :C]
        bias[:, 2 * s + 1] = i['gate_bias'][s, C:]

    inv = 1.0 / np.sqrt(np.float32(C))
    lin = np.concatenate([i['lin_w_0e'] * inv, i['lin_w_1o'] * inv], axis=1)  # [128, 256]

    ident = np.eye(128, dtype=np.float32)

    wblk_bf = _bf16(wblk); gk_bf = _bf16(gk); lin_bf = _bf16(lin); ident_bf = _bf16(ident)

    in_maps = []
    for k in range(N_CORES):
        rows = device_rows[k]
        xr_core = np.zeros((NODES_PER_CORE, C * D), np.float32)
        valid = rows >= 0
        xr_core[valid] = xr[rows[valid]].reshape(-1, C * D)
        in_maps.append({
            'xr': _bf16(xr_core),
            'wblk': wblk_bf,
            'gk': gk_bf,
            'bias': bias,
            'lin': lin_bf,
            'ident': ident_bf,
            'identf': ident,
        })
    return in_maps, device_rows, overflow_idx


# ----------------------------------------------------------------------------
# device kernel
# ----------------------------------------------------------------------------

def build_device(repeat=1, stages=5):
    import concourse.bacc as bacc
    import concourse.mybir as mybir
    from concourse.tile import TileContext

    f32, bf16 = mybir.dt.float32, mybir.dt.bfloat16
    AL = mybir.AluOpType

    nc = bacc.Bacc("TRN2", target_bir_lowering=False, debug=False,
                   num_devices=N_CORES)

    xr_d = nc.dram_tensor('xr', [NODES_PER_CORE, C * D], bf16, kind='ExternalInput').ap()
    wblk_d = nc.dram_tensor('wblk', [KP, S * NPAIR * 72], bf16, kind='ExternalInput').ap()
    gk_d = nc.dram_tensor('gk', [C, S * 2 * C], bf16, kind='ExternalInput').ap()
    bias_d = nc.dram_tensor('bias', [C, S * 2], f32, kind='ExternalInput').ap()
    lin_d = nc.dram_tensor('lin', [C, 2 * C], bf16, kind='ExternalInput').ap()
    ident_d = nc.dram_tensor('ident', [C, C], bf16, kind='ExternalInput').ap()
    identf_d = nc.dram_tensor('identf', [C, C], f32, kind='ExternalInput').ap()
    out_d = nc.dram_tensor('out', [NODES_PER_CORE, 4 * C], f32, kind='ExternalOutput').ap()

    with TileContext(nc) as tc:
        with (
            tc.tile_pool(name='const', bufs=1) as constp,
            tc.tile_pool(name='xr', bufs=2) as xrp,
            tc.tile_pool(name='yx', bufs=2) as yxp,
            tc.tile_pool(name='v', bufs=4) as vp,
            tc.tile_pool(name='sb', bufs=3) as sbp,
            tc.tile_pool(name='facc', bufs=2) as faccp,
            tc.tile_pool(name='outt', bufs=2) as outp,
            tc.tile_pool(name='ps_t', bufs=2, space='PSUM') as ps_t,
            tc.tile_pool(name='ps_ft', bufs=2, space='PSUM') as ps_ft,
            tc.tile_pool(name='ps_misc', bufs=4, space='PSUM') as ps_m,
        ):
            wblk_s = constp.tile([KP, S * NPAIR * 72], bf16)
            nc.sync.dma_start(out=wblk_s[:], in_=wblk_d[:])
            gk_s = constp.tile([C, S * 2 * C], bf16)
            nc.sync.dma_start(out=gk_s[:], in_=gk_d[:])
            bias_s = constp.tile([C, S * 2], f32)
            nc.sync.dma_start(out=bias_s[:], in_=bias_d[:])
            lin_s = constp.tile([C, 2 * C], bf16)
            nc.sync.dma_start(out=lin_s[:], in_=lin_d[:])
            ident_s = constp.tile([C, C], bf16)
            nc.sync.dma_start(out=ident_s[:], in_=ident_d[:])
            identf_s = constp.tile([C, C], f32)
            nc.sync.dma_start(out=identf_s[:], in_=identf_d[:])

            for rep in range(repeat):
                for t in range(TPC):
                    s = t  # species == tile index
                    xr_t = xrp.tile([TILE_N, C * D], bf16)
                    nc.sync.dma_start(out=xr_t[:], in_=xr_d[t * TILE_N:(t + 1) * TILE_N, :])

                    # ---- build YX [128, (c,56)] ----
                    yx = yxp.tile([TILE_N, C * NROW], bf16)
                    if stages < 2:
                        ot = outp.tile([TILE_N, 4 * C], f32)
                        nc.vector.tensor_copy(ot[:, 0:C], xr_t[:, 0:C])
                        nc.vector.memset(ot[:, C:], 0.0)
                        nc.sync.dma_start(out=out_d[t * TILE_N:(t + 1) * TILE_N, :], in_=ot[:])
                        continue
                    # x slots (45..53): strided copy
                    nc.vector.tensor_copy(
                        yx[:, :].rearrange('p (c r) -> p c r', r=NROW)[:, :, 45:54],
                        xr_t[:, :].rearrange('p (c d) -> p c d', d=D))
                    # ones + pad slots
                    yx_cr = yx[:, :].rearrange('p (c r) -> p c r', r=NROW)
                    nc.vector.memset(yx_cr[:, :, 54:55], 1.0)
                    nc.vector.memset(yx_cr[:, :, 55:56], 0.0)
                    # y slots: 9 multiplies, one per diagonal offset o (all step-1)
                    xr_cd = xr_t[:, :].rearrange('p (c d) -> p c d', d=D)
                    for o in range(D):
                        nj = D - o
                        out_ap = yx_cr[:, :, OSTART[o]:OSTART[o] + nj]
                        in0 = xr_cd[:, :, 0:nj]
                        in1 = xr_cd[:, :, o:o + nj]
                        eng = nc.gpsimd if o >= 7 else nc.vector
                        eng.tensor_mul(out_ap, in0, in1)

                    if stages < 3:
                        ot = outp.tile([TILE_N, 4 * C], f32)
                        nc.vector.tensor_copy(ot[:, 0:C], yx[:, 0:C])
                        nc.vector.memset(ot[:, C:], 0.0)
                        nc.sync.dma_start(out=out_d[t * TILE_N:(t + 1) * TILE_N, :], in_=ot[:])
                        continue

                    # ---- per-pair transpose + matmul; per-4-pair Fx+reduce ----
                    facc = faccp.tile([TILE_N, 4 * C], f32)
                    skip_fx = stages < 4
                    for g in range(NPAIR // 4):
                        ft = ps_ft.tile([TILE_N, 4 * 72], f32)
                        pt = ps_t.tile([KP, 4 * TILE_N], bf16, tag='ptrans')
                        for pl in range(4):
                            p = 4 * g + pl
                            nc.tensor.transpose(pt[:, pl * TILE_N:(pl + 1) * TILE_N],
                                                yx[:, p * KP:(p + 1) * KP], ident_s[:])
                        v = vp.tile([KP, 4 * TILE_N], bf16)
                        eng = nc.scalar if g % 2 == 0 else nc.vector
                        eng.tensor_copy(v[:], pt[:]) if eng is nc.vector else eng.copy(v[:], pt[:])
                        for pl in range(4):
                            p = 4 * g + pl
                            nc.tensor.matmul(
                                ft[:, pl * 72:(pl + 1) * 72],
                                v[:, pl * TILE_N:(pl + 1) * TILE_N],
                                wblk_s[:, (s * NPAIR + p) * 72:(s * NPAIR + p + 1) * 72],
                                start=True, stop=True)
                        if skip_fx:
                            continue
                        # multiply by x and segment-reduce over i(9)
                        # x AP: (pair:4, step 18)(cL:2, step 9)(z:4, step 0)(i:9, step 1)
                        pg = sbp.tile([TILE_N, 4 * 72], bf16, tag='pg')
                        xb4 = xr_t[:, :].rearrange('p (gg pr cl d) -> p gg pr cl d',
                                                   gg=16, pr=4, cl=2)[:, g]  # [128,4,2,9]
                        xb4 = xb4.unsqueeze(3).broadcast_to([TILE_N, 4, 2, 4, D])
                        nc.vector.tensor_mul(
                            pg[:, :].rearrange('p (a b c dd) -> p a b c dd', a=4, b=2, c=4),
                            ft[:, :].rearrange('p (a b c dd) -> p a b c dd', a=4, b=2, c=4),
                            xb4)
                        nc.vector.tensor_reduce(
                            facc[:, 32 * g:32 * (g + 1)],
                            pg[:, :].rearrange('p (a b) -> p a b', b=D),
                            axis=mybir.AxisListType.X, op=AL.add)

                    if stages < 4:
                        ot = outp.tile([TILE_N, 4 * C], f32)
                        nc.vector.tensor_copy(ot[:, 0:256], ft[:, 0:256])
                        nc.vector.memset(ot[:, 256:], 0.0)
                        nc.sync.dma_start(out=out_d[t * TILE_N:(t + 1) * TILE_N, :], in_=ot[:])
                        continue
                    if stages < 5:
                        ot = outp.tile([TILE_N, 4 * C], f32)
                        nc.vector.tensor_copy(ot[:], facc[:])
                        nc.sync.dma_start(out=out_d[t * TILE_N:(t + 1) * TILE_N, :], in_=ot[:])
                        continue

                    # ---- gate ----
                    f_cz = facc[:, :].rearrange('p (c z) -> p c z', z=4)
                    sct = ps_m.tile([C, TILE_N], f32, tag='misc')
                    nc.tensor.transpose(sct[:], f_cz[:, :, 0], identf_s[:])
                    scs = sbp.tile([C, TILE_N], bf16, tag='scs')
                    nc.scalar.copy(scs[:], sct[:])
                    g0 = ps_m.tile([C, TILE_N], f32, tag='misc')
                    g1 = ps_m.tile([C, TILE_N], f32, tag='misc')
                    nc.tensor.matmul(g0[:], gk_s[:, s * 256:s * 256 + 128], scs[:],
                                     start=True, stop=True)
                    nc.tensor.matmul(g1[:], gk_s[:, s * 256 + 128:s * 256 + 256], scs[:],
                                     start=True, stop=True)
                    g0s = sbp.tile([C, TILE_N], f32, tag='g0s')
                    g1s = sbp.tile([C, TILE_N], f32, tag='g1s')
                    nc.vector.tensor_scalar(out=g0s[:], in0=g0[:],
                                            scalar1=bias_s[:, 2 * s:2 * s + 1],
                                            scalar2=None, op0=AL.add)
                    nc.vector.tensor_scalar(out=g1s[:], in0=g1[:],
                                            scalar1=bias_s[:, 2 * s + 1:2 * s + 2],
                                            scalar2=None, op0=AL.add)
                    gt0 = ps_m.tile([TILE_N, C], f32, tag='misc')
                    gt1 = ps_m.tile([TILE_N, C], f32, tag='misc')
                    nc.tensor.transpose(gt0[:], g0s[:], identf_s[:])
                    nc.tensor.transpose(gt1[:], g1s[:], identf_s[:])
                    gts = sbp.tile([TILE_N, 2 * C], f32, tag='gts')
                    nc.scalar.copy(gts[:, 0:C], gt0[:])
                    nc.scalar.copy(gts[:, C:2 * C], gt1[:])

                    # ---- apply gates ----
                    fg = sbp.tile([TILE_N, 4 * C], bf16, tag='fg')
                    fg_cz = fg[:, :].rearrange('p (c z) -> p c z', z=4)
                    nc.vector.tensor_mul(fg_cz[:, :, 0], f_cz[:, :, 0], gts[:, 0:C])
                    nc.vector.tensor_mul(
                        fg_cz[:, :, 1:4], f_cz[:, :, 1:4],
                        gts[:, C:2 * C].unsqueeze(2).broadcast_to([TILE_N, C, 3]))

                    # ---- linear ----
                    ot = outp.tile([TILE_N, 4 * C], f32)
                    for z in range(4):
                        fgt = ps_m.tile([C, TILE_N], bf16, tag='misc')
                        nc.tensor.transpose(fgt[:], fg_cz[:, :, z], ident_s[:])
                        fgts = sbp.tile([C, TILE_N], bf16, tag='fgts')
                        nc.scalar.copy(fgts[:], fgt[:])
                        oz = ps_m.tile([C, TILE_N], f32, tag='misc')
                        lsl = lin_s[:, 0:C] if z == 0 else lin_s[:, C:2 * C]
                        nc.tensor.matmul(oz[:], lsl, fgts[:], start=True, stop=True)
                        ozs = sbp.tile([C, TILE_N], f32, tag='ozs')
                        nc.scalar.copy(ozs[:], oz[:])
                        ozt = ps_m.tile([TILE_N, C], f32, tag='misc')
                        nc.tensor.transpose(ozt[:], ozs[:], identf_s[:])
                        if z == 0:
                            nc.vector.tensor_copy(ot[:, 0:C], ozt[:])
                        else:
                            nc.vector.tensor_copy(
                                ot[:, C:].rearrange('p (c zz) -> p c zz', zz=3)[:, :, z - 1],
                                ozt[:])

                    nc.sync.dma_start(out=out_d[t * TILE_N:(t + 1) * TILE_N, :], in_=ot[:])

    nc.compile()
    return nc


_NC_CACHE = {}


def _get_device(repeat=1, stages=5):
    key = (repeat, stages)
    if key not in _NC_CACHE:
        _NC_CACHE[key] = build_device(repeat, stages)
    return _NC_CACHE[key]


def kernel(**inputs):
    from concourse.bass_utils import run_bass_kernel_spmd

    in_maps, device_rows, overflow_idx = host_prepare(inputs)
    nc = _get_device(1)
    res = run_bass_kernel_spmd(nc, in_maps, list(range(N_CORES)))

    ntot = np.asarray(inputs['node_species']).shape[0]
    out = np.zeros((ntot, 4 * C), np.float32)
    for k in range(N_CORES):
        rows = device_rows[k]
        valid = rows >= 0
        out[rows[valid]] = res.results[k]['out'][valid]
    if len(overflow_idx):
        out[overflow_idx] = _numpy_forward(inputs, overflow_idx)
    return out



# revision 3
# speedup vs baseline: 29.5024x; 29.5024x over previous
"""Trainium2 Bass kernel for nn_EquivariantProductBasisBlock (MACE product-basis block).

Self-contained: host-side sharding/preprocessing + Bass/Tile device kernel on 8 cores.

Math (validated vs reference): per node n, channel c, species s, x = x[n,c,:] in R^9:
    out[z] = sum_i x_i * F[z,i],   F = C1 + C2 @ x + C3h @ y,   y = {x_j x_k}_{j<=k}
with C* the species/channel coefficient tables folded from (u*, w*) on the host.
Then gate = f0 @ gate_kernel[s] + gate_bias[s]; f0*=gate[:C]; f1*=gate[C:];
out = [f0 @ lin0, f1 @ lin1] / sqrt(C).

Device layout: nodes on SBUF partitions (128/tile), species-sorted so every tile is
species-pure; schedule identical across cores (tile t = species t, 10 tiles/core).
Per tile: DVE builds monomial slots YX[128n,(c,56)]; PE transposes each channel-pair
slice -> V[112,128]; matmul(lhsT=V, rhs=Wblk[s,pair]) -> F^T[128n,72] in PSUM;
DVE multiplies by x (broadcast AP) and segment-reduces -> f[128n,(c,4)];
then gate matmul + gating + linear + output transposes.

Species overflow beyond 1024 nodes/species (and any species with >1024 nodes) is
computed on the host in numpy and merged (tiny for the target distribution).
"""

import numpy as np

N_CORES = 8
C, D, S = 128, 9, 10
NM = 45           # deg-2 monomials
NROW = 56         # per-channel slots: 45 y | 9 x | 1 one | 1 pad
KP = 2 * NROW     # 112 rows per channel-pair
NPAIR = C // 2    # 64
TILE_N = 128
TPC = S           # tiles per core (one per species)
NODES_PER_CORE = TPC * TILE_N          # 1280
CAP_PER_SPECIES = N_CORES * TILE_N     # 1024 device-handled nodes per species

# monomials ordered by diagonal offset o=k-j then j: slot(o,j) = OSTART[o]+j.
# This makes each device y-build op pure step-1 (out/in0/in1 all contiguous runs).
OSTART = [0] * D
for o in range(1, D):
    OSTART[o] = OSTART[o - 1] + (D - (o - 1))
MONO_JK = [(j, j + o) for o in range(D) for j in range(D - o)]


# ----------------------------------------------------------------------------
# host math
# ----------------------------------------------------------------------------

def _build_xr(node_feats):
    n = node_feats.shape[0]
    x = np.empty((n, C, D), np.float32)
    x[:, :, 0] = node_feats[:, :C]
    x[:, :, 1:4] = node_feats[:, C:4 * C].reshape(n, C, 3)
    x[:, :, 4:9] = node_feats[:, 4 * C:].reshape(n, C, 5)
    return x


def _build_coeff_tables(i):
    def c3h(u3, w3):
        c3 = np.einsum('zijkp,spc->sczijk', u3, w3[:, :, :], optimize=True)
        out = np.zeros(c3.shape[:4] + (NM,), np.float64)
        for m, (j, k) in enumerate(MONO_JK):
            out[..., m] = c3[..., j, k] if j == k else c3[..., j, k] + c3[..., k, j]
        return out

    def c2(u2, w2):
        return np.einsum('zijp,spc->sczij', u2, w2, optimize=True)

    def c1(u1, w1):
        return np.einsum('zip,spc->sczi', u1, w1, optimize=True)

    h0 = c3h(i['u3_0e'], i['w3_0e']); h1 = c3h(i['u3_1o'], i['w3_1o'])
    q0 = c2(i['u2_0e'], i['w2_0e']);  q1 = c2(i['u2_1o'], i['w2_1o'])
    l0 = c1(i['u1_0e'], i['w1_0e']);  l1 = c1(i['u1_1o'], i['w1_1o'])

    W = np.zeros((S, C, NROW, 36), np.float64)
    W[:, :, 0:45, 0:9] = np.moveaxis(h0[:, :, 0], -1, -2)
    W[:, :, 45:54, 0:9] = np.moveaxis(q0[:, :, 0], -1, -2)
    W[:, :, 54, 0:9] = l0[:, :, 0]
    for z in range(3):
        sl = slice(9 + z * 9, 18 + z * 9)
        W[:, :, 0:45, sl] = np.moveaxis(h1[:, :, z], -1, -2)
        W[:, :, 45:54, sl] = np.moveaxis(q1[:, :, z], -1, -2)
        W[:, :, 54, sl] = l1[:, :, z]
    return W.astype(np.float32)   # [S, C, 56, 36]


def _numpy_forward(inputs, idx):
    """Reference-equivalent host computation for node subset idx (overflow path)."""
    i = {k: np.asarray(v) for k, v in inputs.items()}
    nf = i['node_feats'][idx]; sp = i['node_species'][idx]
    xr = _build_xr(nf)
    W = _build_coeff_tables(i)
    n = nf.shape[0]
    V = np.empty((n, C, NROW), np.float32)
    for m, (j, k) in enumerate(MONO_JK):
        V[:, :, m] = xr[:, :, j] * xr[:, :, k]
    V[:, :, 45:54] = xr
    V[:, :, 54] = 1.0
    V[:, :, 55] = 0.0
    F = np.einsum('ncm,ncmz->ncz', V, W[sp], optimize=True)
    f = np.einsum('nczi,nci->ncz', F.reshape(n, C, 4, D), xr, optimize=True)
    f0, f1 = f[:, :, 0], f[:, :, 1:4]
    gate = np.einsum('nc,nck->nk', f0, i['gate_kernel'][sp], optimize=True) + i['gate_bias'][sp]
    f0g = f0 * gate[:, :C]
    f1g = f1 * gate[:, C:, None]
    inv = 1.0 / np.sqrt(np.float32(C))
    o0 = np.einsum('nc,ck->nk', f0g, i['lin_w_0e'], optimize=True) * inv
    o1 = np.einsum('ncd,ck->nkd', f1g, i['lin_w_1o'], optimize=True) * inv
    return np.concatenate([o0.reshape(n, C), o1.reshape(n, C * 3)], axis=1).astype(np.float32)


def _bf16(x):
    import ml_dtypes
    return np.asarray(x, np.float32).astype(ml_dtypes.bfloat16)


def host_prepare(inputs):
    """Returns (per_core_inmaps, scatter_idx [N_CORES,1280] global node ids (-1 pad),
    overflow_idx)."""
    i = {k: np.asarray(v) for k, v in inputs.items()}
    sp = i['node_species']
    ntot = sp.shape[0]

    order = np.argsort(sp, kind='stable')
    sorted_sp = sp[order]
    device_rows = np.full((N_CORES, NODES_PER_CORE), -1, np.int64)
    overflow = []
    for s in range(S):
        ids = order[sorted_sp == s]
        dev = ids[:CAP_PER_SPECIES]
        overflow.append(ids[CAP_PER_SPECIES:])
        for k in range(N_CORES):
            chunk = dev[k * TILE_N:(k + 1) * TILE_N]
            device_rows[k, s * TILE_N: s * TILE_N + len(chunk)] = chunk
    overflow_idx = np.concatenate(overflow) if overflow else np.zeros(0, np.int64)

    xr = _build_xr(i['node_feats'])                       # [N, C, 9]
    W = _build_coeff_tables(i)                            # [S, C, 56, 36]

    # block-diag pair table: [112 rows, S*64*72 cols] (row-major per pair block)
    wblk = np.zeros((KP, S * NPAIR * 72), np.float32)
    for s in range(S):
        for p in range(NPAIR):
            base = (s * NPAIR + p) * 72
            wblk[0:NROW, base:base + 36] = W[s, 2 * p]
            wblk[NROW:KP, base + 36:base + 72] = W[s, 2 * p + 1]

    gk = np.zeros((C, S * 2 * C), np.float32)             # rows c, col s*256 + j
    for s in range(S):
        gk[:, s * 256:(s + 1) * 256] = i['gate_kernel'][s]

    bias = np.zeros((C, S * 2), np.float32)               # rows k2%128, col s*2 + half
    for s in range(S):
        bias[:, 2 * s] = i['gate_bias'][s, :C]
        bias[:, 2 * s + 1] = i['gate_bias'][s, C:]

    inv = 1.0 / np.sqrt(np.float32(C))
    lin = np.concatenate([i['lin_w_0e'] * inv, i['lin_w_1o'] * inv], axis=1)  # [128, 256]

    ident = np.eye(128, dtype=np.float32)

    wblk_bf = _bf16(wblk); gk_bf = _bf16(gk); lin_bf = _bf16(lin); ident_bf = _bf16(ident)

    in_maps = []
    for k in range(N_CORES):
        rows = device_rows[k]
        xr_core = np.zeros((NODES_PER_CORE, C * D), np.float32)
        valid = rows >= 0
        xr_core[valid] = xr[rows[valid]].reshape(-1, C * D)
        in_maps.append({
            'xr': _bf16(xr_core),
            'wblk': wblk_bf,
            'gk': gk_bf,
            'bias': bias,
            'lin': lin_bf,
            'ident': ident_bf,
            'identf': ident,
        })
    return in_maps, device_rows, overflow_idx


# ----------------------------------------------------------------------------
# device kernel
# ----------------------------------------------------------------------------

def build_device(repeat=1, stages=5):
    import concourse.bacc as bacc
    import concourse.mybir as mybir
    from concourse.tile import TileContext

    f32, bf16 = mybir.dt.float32, mybir.dt.bfloat16
    AL = mybir.AluOpType

    nc = bacc.Bacc("TRN2", target_bir_lowering=False, debug=False,
                   num_devices=N_CORES)

    xr_d = nc.dram_tensor('xr', [NODES_PER_CORE, C * D], bf16, kind='ExternalInput').ap()
    wblk_d = nc.dram_tensor('wblk', [KP, S * NPAIR * 72], bf16, kind='ExternalInput').ap()
    gk_d = nc.dram_tensor('gk', [C, S * 2 * C], bf16, kind='ExternalInput').ap()
    bias_d = nc.dram_tensor('bias', [C, S * 2], f32, kind='ExternalInput').ap()
    lin_d = nc.dram_tensor('lin', [C, 2 * C], bf16, kind='ExternalInput').ap()
    ident_d = nc.dram_tensor('ident', [C, C], bf16, kind='ExternalInput').ap()
    identf_d = nc.dram_tensor('identf', [C, C], f32, kind='ExternalInput').ap()
    out_d = nc.dram_tensor('out', [NODES_PER_CORE, 4 * C], f32, kind='ExternalOutput').ap()

    with TileContext(nc) as tc:
        with (
            tc.tile_pool(name='const', bufs=1) as constp,
            tc.tile_pool(name='xr', bufs=2) as xrp,
            tc.tile_pool(name='yx', bufs=2) as yxp,
            tc.tile_pool(name='v', bufs=4) as vp,
            tc.tile_pool(name='sb', bufs=3) as sbp,
            tc.tile_pool(name='facc', bufs=2) as faccp,
            tc.tile_pool(name='outt', bufs=2) as outp,
            tc.tile_pool(name='ps_t', bufs=2, space='PSUM') as ps_t,
            tc.tile_pool(name='ps_ft', bufs=2, space='PSUM') as ps_ft,
            tc.tile_pool(name='ps_misc', bufs=4, space='PSUM') as ps_m,
        ):
            wblk_s = constp.tile([KP, S * NPAIR * 72], bf16)
            nc.sync.dma_start(out=wblk_s[:], in_=wblk_d[:])
            gk_s = constp.tile([C, S * 2 * C], bf16)
            nc.sync.dma_start(out=gk_s[:], in_=gk_d[:])
            bias_s = constp.tile([C, S * 2], f32)
            nc.sync.dma_start(out=bias_s[:], in_=bias_d[:])
            lin_s = constp.tile([C, 2 * C], bf16)
            nc.sync.dma_start(out=lin_s[:], in_=lin_d[:])
            ident_s = constp.tile([C, C], bf16)
            nc.sync.dma_start(out=ident_s[:], in_=ident_d[:])
            identf_s = constp.tile([C, C], f32)
            nc.sync.dma_start(out=identf_s[:], in_=identf_d[:])

            def one_pass():
                for t in range(TPC):
                    s = t  # species == tile index
                    xr_t = xrp.tile([TILE_N, C * D], bf16)
                    nc.sync.dma_start(out=xr_t[:], in_=xr_d[t * TILE_N:(t + 1) * TILE_N, :])

                    # ---- build YX [128, (c,56)] ----
                    yx = yxp.tile([TILE_N, C * NROW], bf16)
                    if stages < 2:
                        ot = outp.tile([TILE_N, 4 * C], f32)
                        nc.vector.tensor_copy(ot[:, 0:C], xr_t[:, 0:C])
                        nc.vector.memset(ot[:, C:], 0.0)
                        nc.sync.dma_start(out=out_d[t * TILE_N:(t + 1) * TILE_N, :], in_=ot[:])
                        continue
                    # x slots (45..53): strided copy
                    nc.vector.tensor_copy(
                        yx[:, :].rearrange('p (c r) -> p c r', r=NROW)[:, :, 45:54],
                        xr_t[:, :].rearrange('p (c d) -> p c d', d=D))
                    # ones + pad slots
                    yx_cr = yx[:, :].rearrange('p (c r) -> p c r', r=NROW)
                    nc.vector.memset(yx_cr[:, :, 54:55], 1.0)
                    nc.vector.memset(yx_cr[:, :, 55:56], 0.0)
                    # y slots: 9 multiplies, one per diagonal offset o (all step-1)
                    xr_cd = xr_t[:, :].rearrange('p (c d) -> p c d', d=D)
                    for o in range(D):
                        nj = D - o
                        out_ap = yx_cr[:, :, OSTART[o]:OSTART[o] + nj]
                        in0 = xr_cd[:, :, 0:nj]
                        in1 = xr_cd[:, :, o:o + nj]
                        eng = nc.gpsimd if o >= 7 else nc.vector
                        eng.tensor_mul(out_ap, in0, in1)

                    if stages < 3:
                        ot = outp.tile([TILE_N, 4 * C], f32)
                        nc.vector.tensor_copy(ot[:, 0:C], yx[:, 0:C])
                        nc.vector.memset(ot[:, C:], 0.0)
                        nc.sync.dma_start(out=out_d[t * TILE_N:(t + 1) * TILE_N, :], in_=ot[:])
                        continue

                    # ---- per-pair transpose + matmul; per-4-pair Fx+reduce ----
                    facc = faccp.tile([TILE_N, 4 * C], f32)
                    skip_fx = stages < 4
                    for g in range(NPAIR // 4):
                        ft = ps_ft.tile([TILE_N, 4 * 72], f32)
                        pt = ps_t.tile([KP, 4 * TILE_N], bf16, tag='ptrans')
                        for pl in range(4):
                            p = 4 * g + pl
                            nc.tensor.transpose(pt[:, pl * TILE_N:(pl + 1) * TILE_N],
                                                yx[:, p * KP:(p + 1) * KP], ident_s[:])
                        v = vp.tile([KP, 4 * TILE_N], bf16)
                        eng = nc.scalar if g % 2 == 0 else nc.vector
                        eng.tensor_copy(v[:], pt[:]) if eng is nc.vector else eng.copy(v[:], pt[:])
                        for pl in range(4):
                            p = 4 * g + pl
                            nc.tensor.matmul(
                                ft[:, pl * 72:(pl + 1) * 72],
                                v[:, pl * TILE_N:(pl + 1) * TILE_N],
                                wblk_s[:, (s * NPAIR + p) * 72:(s * NPAIR + p + 1) * 72],
                                start=True, stop=True)
                        if skip_fx:
                            continue
                        # multiply by x and segment-reduce over i(9)
                        # x AP: (pair:4, step 18)(cL:2, step 9)(z:4, step 0)(i:9, step 1)
                        pg = sbp.tile([TILE_N, 4 * 72], bf16, tag='pg')
                        xb4 = xr_t[:, :].rearrange('p (gg pr cl d) -> p gg pr cl d',
                                                   gg=16, pr=4, cl=2)[:, g]  # [128,4,2,9]
                        xb4 = xb4.unsqueeze(3).broadcast_to([TILE_N, 4, 2, 4, D])
                        nc.vector.tensor_mul(
                            pg[:, :].rearrange('p (a b c dd) -> p a b c dd', a=4, b=2, c=4),
                            ft[:, :].rearrange('p (a b c dd) -> p a b c dd', a=4, b=2, c=4),
                            xb4)
                        nc.vector.tensor_reduce(
                            facc[:, 32 * g:32 * (g + 1)],
                            pg[:, :].rearrange('p (a b) -> p a b', b=D),
                            axis=mybir.AxisListType.X, op=AL.add)

                    if stages < 4:
                        ot = outp.tile([TILE_N, 4 * C], f32)
                        nc.vector.tensor_copy(ot[:, 0:256], ft[:, 0:256])
                        nc.vector.memset(ot[:, 256:], 0.0)
                        nc.sync.dma_start(out=out_d[t * TILE_N:(t + 1) * TILE_N, :], in_=ot[:])
                        continue
                    if stages < 5:
                        ot = outp.tile([TILE_N, 4 * C], f32)
                        nc.vector.tensor_copy(ot[:], facc[:])
                        nc.sync.dma_start(out=out_d[t * TILE_N:(t + 1) * TILE_N, :], in_=ot[:])
                        continue

                    # ---- gate ----
                    f_cz = facc[:, :].rearrange('p (c z) -> p c z', z=4)
                    sct = ps_m.tile([C, TILE_N], f32, tag='misc')
                    nc.tensor.transpose(sct[:], f_cz[:, :, 0], identf_s[:])
                    scs = sbp.tile([C, TILE_N], bf16, tag='scs')
                    nc.scalar.copy(scs[:], sct[:])
                    g0 = ps_m.tile([C, TILE_N], f32, tag='misc')
                    g1 = ps_m.tile([C, TILE_N], f32, tag='misc')
                    nc.tensor.matmul(g0[:], gk_s[:, s * 256:s * 256 + 128], scs[:],
                                     start=True, stop=True)
                    nc.tensor.matmul(g1[:], gk_s[:, s * 256 + 128:s * 256 + 256], scs[:],
                                     start=True, stop=True)
                    g0s = sbp.tile([C, TILE_N], f32, tag='g0s')
                    g1s = sbp.tile([C, TILE_N], f32, tag='g1s')
                    nc.vector.tensor_scalar(out=g0s[:], in0=g0[:],
                                            scalar1=bias_s[:, 2 * s:2 * s + 1],
                                            scalar2=None, op0=AL.add)
                    nc.vector.tensor_scalar(out=g1s[:], in0=g1[:],
                                            scalar1=bias_s[:, 2 * s + 1:2 * s + 2],
                                            scalar2=None, op0=AL.add)
                    gt0 = ps_m.tile([TILE_N, C], f32, tag='misc')
                    gt1 = ps_m.tile([TILE_N, C], f32, tag='misc')
                    nc.tensor.transpose(gt0[:], g0s[:], identf_s[:])
                    nc.tensor.transpose(gt1[:], g1s[:], identf_s[:])
                    gts = sbp.tile([TILE_N, 2 * C], f32, tag='gts')
                    nc.scalar.copy(gts[:, 0:C], gt0[:])
                    nc.scalar.copy(gts[:, C:2 * C], gt1[:])

                    # ---- apply gates ----
                    fg = sbp.tile([TILE_N, 4 * C], bf16, tag='fg')
                    fg_cz = fg[:, :].rearrange('p (c z) -> p c z', z=4)
                    nc.vector.tensor_mul(fg_cz[:, :, 0], f_cz[:, :, 0], gts[:, 0:C])
                    nc.vector.tensor_mul(
                        fg_cz[:, :, 1:4], f_cz[:, :, 1:4],
                        gts[:, C:2 * C].unsqueeze(2).broadcast_to([TILE_N, C, 3]))

                    # ---- linear ----
                    ot = outp.tile([TILE_N, 4 * C], f32)
                    for z in range(4):
                        fgt = ps_m.tile([C, TILE_N], bf16, tag='misc')
                        nc.tensor.transpose(fgt[:], fg_cz[:, :, z], ident_s[:])
                        fgts = sbp.tile([C, TILE_N], bf16, tag='fgts')
                        nc.scalar.copy(fgts[:], fgt[:])
                        oz = ps_m.tile([C, TILE_N], f32, tag='misc')
                        lsl = lin_s[:, 0:C] if z == 0 else lin_s[:, C:2 * C]
                        nc.tensor.matmul(oz[:], lsl, fgts[:], start=True, stop=True)
                        ozs = sbp.tile([C, TILE_N], f32, tag='ozs')
                        nc.scalar.copy(ozs[:], oz[:])
                        ozt = ps_m.tile([TILE_N, C], f32, tag='misc')
                        nc.tensor.transpose(ozt[:], ozs[:], identf_s[:])
                        if z == 0:
                            nc.vector.tensor_copy(ot[:, 0:C], ozt[:])
                        else:
                            nc.vector.tensor_copy(
                                ot[:, C:].rearrange('p (c zz) -> p c zz', zz=3)[:, :, z - 1],
                                ozt[:])

                    nc.sync.dma_start(out=out_d[t * TILE_N:(t + 1) * TILE_N, :], in_=ot[:])

            if repeat == 1:
                one_pass()
            else:
                # hardware loop: program size stays constant in `repeat`, so the
                # repeat-slope measures actual device execution per pass.
                ET = mybir.EngineType
                with tc.For_i(0, repeat, 1,
                              hint_engines=(ET.PE, ET.DVE, ET.Activation,
                                            ET.Pool, ET.SP)):
                    one_pass()

    nc.compile()
    return nc


_NC_CACHE = {}


def _get_device(repeat=1, stages=5):
    key = (repeat, stages)
    if key not in _NC_CACHE:
        _NC_CACHE[key] = build_device(repeat, stages)
    return _NC_CACHE[key]


def kernel(**inputs):
    from concourse.bass_utils import run_bass_kernel_spmd

    in_maps, device_rows, overflow_idx = host_prepare(inputs)
    nc = _get_device(1)
    res = run_bass_kernel_spmd(nc, in_maps, list(range(N_CORES)))

    ntot = np.asarray(inputs['node_species']).shape[0]
    out = np.zeros((ntot, 4 * C), np.float32)
    for k in range(N_CORES):
        rows = device_rows[k]
        valid = rows >= 0
        out[rows[valid]] = res.results[k]['out'][valid]
    if len(overflow_idx):
        out[overflow_idx] = _numpy_forward(inputs, overflow_idx)
    return out

